# revision 1
# baseline (speedup 1.0000x reference)
"""MoE block (B=16, C=192, H=W=32, E=8, top-2, 3x3 same-conv experts) on 8 trn2 cores.

Strategy:
  - Router (tiny: pool -> 192x8 matmul -> softmax -> top2) computed on host in numpy.
  - Conv is linear in weights, so the top-2 expert combine folds into ONE conv
    per sample with host-combined weights:
        out[b] = conv(x[b], sum_k w_bk * W_ek) + sum_k w_bk * b_ek
    Device work: 16 convs total -> 2 per core (data-parallel over batch).
  - Each conv = 9 shifted bf16 matmuls (taps) accumulating in fp32 PSUM;
    contract =
    input channels (192 = 128 + 64), M = output channels (192 = 128 + 64),
    N = 512 pixels (half image).
  - PE-array packing: column tiling is rejected by walrus on TRN2, so only
    row tiling is used. Each K=64 leftover-channel tap runs as a row PAIR
    (tile_position rows 0/64) covering BOTH pixel blocks concurrently,
    writing two different PSUM banks. Partitions 64..127 of the TB x-tile and
    of the K64 weight tile hold duplicates of partitions 0..63.
    144 naive matmuls -> 108 PE slots.
"""

import numpy as np

B, C, H, W = 16, 192, 32, 32
E, TOPK = 8, 2
NCORES = 8
S = B // NCORES          # samples per core
PW = W + 2               # padded width 34
PP = PW * PW             # padded pixels 1156
HWP = H * W              # 1024
PBS = 512                # pixels per block
ROWS_PB = 16             # output rows per block
XROWS = [(0, 18), (15, 34)]   # padded-row range each pixel block needs
TAPS = [(t // 3, t % 3) for t in range(9)]
N_WARMUP = 12

_cache = {}


def _build_module():
    import concourse.tile as tile
    from concourse import bacc, mybir

    f32 = mybir.dt.float32
    f32r = mybir.dt.bfloat16  # compute dtype (variable name kept from the f32r variant)

    nc = bacc.Bacc("TRN2", target_bir_lowering=False, debug=False, num_devices=NCORES)
    xp_d = nc.dram_tensor("xp", [S, C, PP], f32r, kind="ExternalInput")
    wa_d = nc.dram_tensor("wa", [S, 128, 9 * C], f32r, kind="ExternalInput")
    wbb_d = nc.dram_tensor("wbb", [S, 64, 9 * C], f32r, kind="ExternalInput")
    bias_d = nc.dram_tensor("bias", [128, 4], f32, kind="ExternalInput")
    out_d = nc.dram_tensor("out", [S, C, HWP], f32, kind="ExternalOutput")

    with tile.TileContext(nc) as tc:
        with (
            tc.tile_pool(name="xin", bufs=1) as xin,
            tc.tile_pool(name="win", bufs=1) as win,
            tc.tile_pool(name="cst", bufs=1) as cst,
            tc.tile_pool(name="ps", bufs=3, space="PSUM") as ps,
            tc.tile_pool(name="pw", bufs=1, space="PSUM") as pw,
            tc.tile_pool(name="oev", bufs=4) as oev,
        ):
            # --- PE warmup: tiny matmuls on zeros keep the clock ramped while
            # input DMAs stream in.
            scr = cst.tile([128, 512], mybir.dt.bfloat16, name="scr", tag="scr")
            nc.vector.memset(scr[:], 0.0)
            ps_scr = pw.tile([128, 512], f32, name="ps_scr", tag="ps_scr")
            for i in range(N_WARMUP):
                nc.tensor.matmul(ps_scr[:], scr[:, 0:128], scr[:], start=True,
                                 stop=True, skip_group_check=True)

            bias_t = cst.tile([128, 4], f32, name="bias_t", tag="bias_t")

            Ta = {}   # (s, pb) -> [128, rows*34] ch0-127 chunk
            TB = {}   # s -> [128, 1156]: ch128-191, duplicated on both halves
            WaC = {}  # (s, c) -> weight chunks for taps 0-2 / 3-8
            WBB = {}  # s -> [128, 9*192] K64 weights, duplicated halves

            def emit_input_dmas(s):
                ta0 = xin.tile([128, 18 * PW], f32r, name=f"Ta{s}_0", tag=f"Ta{s}_0")
                nc.sync.dma_start(ta0[:], xp_d[s, 0:128, 0 : 18 * PW])
                Ta[(s, 0)] = ta0
                # A-block weights in three tap-chunks, interleaved across the
                # ACT and SP issue paths so arrival order matches tap order.
                for ch, eng in ((0, nc.scalar), (1, nc.sync), (2, nc.scalar)):
                    wac = win.tile([128, 3 * C], f32r, name=f"WaC{s}_{ch}",
                                   tag=f"WaC{s}_{ch}")
                    eng.dma_start(wac[:], wa_d[s, :, ch * 3 * C : (ch + 1) * 3 * C])
                    WaC[(s, ch)] = wac

                ta1 = xin.tile([128, 19 * PW], f32r, name=f"Ta{s}_1", tag=f"Ta{s}_1")
                nc.sync.dma_start(ta1[:], xp_d[s, 0:128, 15 * PW : 34 * PW])
                Ta[(s, 1)] = ta1

                # Lower half serves pixel-block-0 windows (padded rows 0..17),
                # upper half serves pixel-block-1 windows (rows 15..33) -- so
                # each half only needs its row range; no duplicate bytes.
                tb = xin.tile([128, PP], f32r, name=f"TB_{s}", tag=f"TB_{s}")
                nc.sync.dma_start(tb[0:64, 0 : 18 * PW], xp_d[s, 128:192, 0 : 18 * PW])
                nc.gpsimd.dma_start(tb[64:128, 15 * PW : PP], xp_d[s, 128:192, 15 * PW : PP])
                TB[s] = tb

                wbb = win.tile([128, 9 * C], f32r, name=f"WBB{s}", tag=f"WBB{s}")
                nc.gpsimd.dma_start(wbb[0:64, :], wbb_d[s])
                nc.gpsimd.dma_start(wbb[64:128, :], wbb[0:64, :])
                WBB[s] = wbb
                if s == 0:
                    nc.sync.dma_start(bias_t[:], bias_d[:])

            def wa_tap(s, t):
                """lhsT slice for tap t: [K=128, M=192] within its chunk."""
                return WaC[(s, t // 3)][:, (t % 3) * C : (t % 3 + 1) * C]

            def ta_rhs(s, pb, t):
                dy, dx = TAPS[t]
                v = Ta[(s, pb)][:].rearrange("p (r c) -> p r c", c=PW)
                y = ROWS_PB * pb - XROWS[pb][0] + dy
                return v[:, y : y + ROWS_PB, dx : dx + W]

            def tb_rhs(s, half, pb, t):
                """K64 moving AP from the duplicated TB tile: partition half
                `half` (0 -> rows 0..63, 1 -> 64..127), tap-t window of pb."""
                dy, dx = TAPS[t]
                v = TB[s][:].rearrange("p (r c) -> p r c", c=PW)
                y = ROWS_PB * pb + dy
                return v[64 * half : 64 * half + 64, y : y + ROWS_PB, dx : dx + W]

            def emit_A(s, pb, psA_pb):
                for t in range(9):
                    nc.tensor.matmul(psA_pb[:], wa_tap(s, t)[:, 0:128],
                                     ta_rhs(s, pb, t), start=(t == 0), stop=False)

            def emit_B(s, psA):
                # tap t covers pixel blocks 0 (array rows 0..63) and 1
                # (rows 64..127) concurrently, into two different PSUM banks.
                for t in range(9):
                    nc.tensor.matmul(psA[0][:], WBB[s][0:64, t * C : t * C + 128],
                                     tb_rhs(s, 0, 0, t),
                                     start=False, stop=(t == 8))
                    nc.tensor.matmul(psA[1][:], WBB[s][64:128, t * C : t * C + 128],
                                     tb_rhs(s, 1, 1, t),
                                     start=False, stop=(t == 8))

            def emit_C(s, pb, psC_pb):
                for t in range(9):
                    nc.tensor.matmul(psC_pb[:], wa_tap(s, t)[:, 128:192],
                                     ta_rhs(s, pb, t), start=(t == 0), stop=False)

            def emit_D(s, psC):
                # row-paired like B: pb0 on rows 0..63, pb1 on rows 64..127.
                for t in range(9):
                    nc.tensor.matmul(psC[0][:], WBB[s][0:64, t * C + 128 : t * C + 192],
                                     tb_rhs(s, 0, 0, t),
                                     start=False, stop=(t == 8))
                    nc.tensor.matmul(psC[1][:], WBB[s][64:128, t * C + 128 : t * C + 192],
                                     tb_rhs(s, 1, 1, t),
                                     start=False, stop=(t == 8))

            def evict_A(s, pb, psA_pb):
                # DVE: PSUM + per-channel bias -> SBUF, then DMA out.
                oA = oev.tile([128, PBS], f32, name=f"oA_{s}_{pb}", tag="oA")
                nc.vector.tensor_scalar_add(oA[:], psA_pb[:], bias_t[:, s : s + 1])
                eng = nc.scalar if s == 0 else nc.sync
                eng.dma_start(out_d[s, 0:128, pb * PBS : (pb + 1) * PBS], oA[:])

            def evict_C(s, psC):
                # ACT: two [64,512] banks -> one [64,1024] tile, one DMA out.
                oC = oev.tile([64, HWP], f32, name=f"oC_{s}", tag="oC")
                for pb in range(2):
                    nc.scalar.activation(
                        oC[:, pb * PBS : (pb + 1) * PBS], psC[pb][:],
                        mybir.ActivationFunctionType.Identity,
                        bias=bias_t[0:64, 2 + s : 3 + s], scale=1.0,
                    )
                eng = nc.gpsimd if s == 0 else nc.sync
                eng.dma_start(out_d[s, 128:192, :], oC[:])

            def emit_sample(s):
                psA = [
                    ps.tile([128, PBS], f32, name=f"psA_{s}_{pb}", tag="psA")
                    for pb in range(2)
                ]
                psC = [
                    ps.tile([64, PBS], f32, name=f"psC_{s}_{pb}", tag="psC")
                    for pb in range(2)
                ]
                emit_A(s, 0, psA[0])
                emit_A(s, 1, psA[1])
                emit_C(s, 0, psC[0])
                emit_C(s, 1, psC[1])
                emit_B(s, psA)
                evict_A(s, 0, psA[0])
                evict_A(s, 1, psA[1])
                emit_D(s, psC)
                evict_C(s, psC)

            emit_input_dmas(0)
            emit_input_dmas(1)
            emit_sample(0)
            emit_sample(1)

    nc.compile()
    return nc


def get_module():
    if "nc" not in _cache:
        _cache["nc"] = _build_module()
    return _cache["nc"]


def _route(x, gate_w, gate_b):
    """Replicates the reference router in numpy fp32. Returns combine [B,E]."""
    pooled = x.mean(axis=(2, 3), dtype=np.float32)
    logits = pooled @ gate_w + gate_b
    z = logits - logits.max(axis=-1, keepdims=True)
    ez = np.exp(z)
    w = ez / ez.sum(axis=-1, keepdims=True)
    topi = np.argsort(-w, axis=-1, kind="stable")[:, :TOPK]
    topw = np.take_along_axis(w, topi, axis=-1)
    topw = topw / (topw.sum(-1, keepdims=True) + 1e-10)
    combine = np.zeros((B, E), np.float32)
    np.put_along_axis(combine, topi, topw, axis=-1)
    return combine


def make_in_maps(x, gate_w, gate_b, expert_w, expert_b):
    x = np.ascontiguousarray(np.asarray(x, np.float32))
    gate_w = np.asarray(gate_w, np.float32)
    gate_b = np.asarray(gate_b, np.float32)
    expert_w = np.asarray(expert_w, np.float32)
    expert_b = np.asarray(expert_b, np.float32)

    combine = _route(x, gate_w, gate_b)                       # [B,E]
    Wc = np.einsum("be,eoikl->boikl", combine, expert_w)      # [B,C,C,3,3]
    bc = combine @ expert_b                                   # [B,C]

    # Padded input images: [B, C, 34*34]
    xp = np.zeros((B, C, PW, PW), np.float32)
    xp[:, :, 1 : H + 1, 1 : W + 1] = x
    xp = xp.reshape(B, C, PP)

    # lhsT layout: WT[b, t, i, o] = Wc[b, o, i, dy, dx]
    WT = Wc.transpose(0, 3, 4, 2, 1).reshape(B, 9, C, C)      # [B, 9, in, out]
    # wa[b, p, t*192+m] = WT[b,t,p,m] for p<128
    wa = np.ascontiguousarray(
        WT[:, :, 0:128, :].transpose(0, 2, 1, 3).reshape(B, 128, 9 * C)
    )
    # K64 weights with duplicated partition halves:
    # wbb[b, p, t*192+m] = WT[b, t, 128 + (p % 64), m]
    wbb = np.ascontiguousarray(
        WT[:, :, 128:192, :].transpose(0, 2, 1, 3).reshape(B, 64, 9 * C)
    )

    import ml_dtypes
    bf16 = ml_dtypes.bfloat16
    xp = xp.astype(bf16)
    wa = wa.astype(bf16)
    wbb = wbb.astype(bf16)

    in_maps = []
    for c in range(NCORES):
        b0 = S * c
        bias = np.zeros((128, 4), np.float32)
        for s in range(S):
            bias[:, s] = bc[b0 + s, 0:128]
            bias[0:64, 2 + s] = bc[b0 + s, 128:192]
        in_maps.append(
            {
                "xp": np.ascontiguousarray(xp[b0 : b0 + S]),
                "wa": np.ascontiguousarray(wa[b0 : b0 + S]),
                "wbb": np.ascontiguousarray(wbb[b0 : b0 + S]),
                "bias": bias,
            }
        )
    return in_maps


def kernel(x, gate_w, gate_b, expert_w, expert_b):
    from concourse.bass_utils import run_bass_kernel_spmd

    nc = get_module()
    in_maps = make_in_maps(x, gate_w, gate_b, expert_w, expert_b)
    res = run_bass_kernel_spmd(nc, in_maps, core_ids=list(range(NCORES)))
    out = np.stack([res.results[c]["out"] for c in range(NCORES)])  # [8,S,C,HWP]
    return out.reshape(B, C, H, W)



# revision 4
# speedup vs baseline: 1.5940x; 1.5940x over previous
"""MoE block (B=16, C=192, H=W=32, E=8, top-2, 3x3 same-conv experts) on 8 trn2 cores.

Strategy (v2, pixel-stationary):
  - Router + top-2 combine computed on host; conv linearity folds the
    expert mix into ONE conv per sample (combined weights). 2 convs/core.
  - Matmul formulation: out[M=128 pixels, N=192 out-ch] with the pixel
    window as the STATIONARY operand and the weights as the MOVING
    operand. Streamed rows per matmul = 192 (vs 512 in the out-ch-major
    form), and M is always a full 128, so total streamed rows hit the
    128x128 packing floor.
  - Contraction (9 taps x 192 in-ch = 1728) is covered by 14 K-chunks
    per (block): 9 full-K chunks for ch 0-127 (one per tap), then
    ch 128-191 packed two-taps-per-chunk using image tiles whose upper
    64 partitions hold the SAME channels pre-shifted by the inter-tap
    pixel offset (d=34 for vertical tap pairs, d=1 horizontal), so one
    AP base serves both halves. 3 pairs via d=34, 1 pair via d=1, plus
    one K=64 single for the leftover tap.
  - 2 samples x 8 pixel-blocks x 14 chunks = 224 matmuls of 192 rows.
  - PE is kept continuously busy from ~0.5us with warmup matmuls sized
    to bridge until the first input DMA lands (an idle gap resets the
    p-state ramp).
  - Input DMAs on SP/ACT (HWDGE) + DVE; output DMAs on Pool (SWDGE) to
    keep HWDGE free; host pre-assembles shifted/duplicated tiles so
    every DMA is a single contiguous transfer.
"""

import numpy as np

B, C, H, W = 16, 192, 32, 32
E, TOPK = 8, 2
NCORES = 8
S = B // NCORES          # samples per core
PW = W + 2               # padded width 34
PP = PW * PW             # padded pixels 1156
NB = 8                   # pixel blocks per sample (4 rows x 32 cols = 128 px)
NCH = 14                 # K-chunks per block
TAPS = [(t // 3, t % 3) for t in range(9)]

# Warmup matmul row-counts (on zeroed SBUF): keep PE busy until the first
# real inputs land. First runs at the LOW p-state, rest at MID.
WARMUP_NS = [192, 192, 192, 192, 192, 192, 192, 192, 128, 128, 128, 128, 128]

_cache = {}


def _build_module():
    import concourse.tile as tile
    from concourse import bacc, mybir

    f32 = mybir.dt.float32
    bf16 = mybir.dt.bfloat16

    nc = bacc.Bacc("TRN2", target_bir_lowering=False, debug=False, num_devices=NCORES)
    ta_d = nc.dram_tensor("ta", [S, 128, PP], bf16, kind="ExternalInput")
    tb34_d = nc.dram_tensor("tb34", [S, 128, PP], bf16, kind="ExternalInput")
    tb1_d = nc.dram_tensor("tb1", [S, 128, PP], bf16, kind="ExternalInput")
    w_d = nc.dram_tensor("w", [S, 128, NCH * C], bf16, kind="ExternalInput")
    out_d = nc.dram_tensor("out", [S, 128, NB, C], f32, kind="ExternalOutput")

    with tile.TileContext(nc) as tc:
        with (
            tc.tile_pool(name="img", bufs=1) as img,
            tc.tile_pool(name="win", bufs=1) as win,
            tc.tile_pool(name="cst", bufs=1) as cst,
            tc.tile_pool(name="ps", bufs=8, space="PSUM") as ps,
            tc.tile_pool(name="oev", bufs=4) as oev,
        ):
            Ta, T34, T1, Wt = {}, {}, {}, {}

            # --- input DMAs -------------------------------------------------
            # Weights on SP, images on ACT: the two HWDGE engines' request
            # streams interleave so transfers arrive in consumption order
            # (w c0-c3, ta, w c4-c8, tb34, w c9-c13, tb1, then sample 1).
            wt0 = win.tile([128, NCH * C], bf16, name="W_0", tag="W_0")
            Wt[0] = wt0
            wt1 = win.tile([128, NCH * C], bf16, name="W_1", tag="W_1")
            Wt[1] = wt1

            nc.sync.dma_start(wt0[:, 0 : 4 * C], w_d[0, :, 0 : 4 * C])
            nc.sync.dma_start(wt0[:, 4 * C : 9 * C], w_d[0, :, 4 * C : 9 * C])
            nc.sync.dma_start(wt0[:, 9 * C : NCH * C], w_d[0, :, 9 * C : NCH * C])
            nc.sync.dma_start(wt1[:, 0 : 9 * C], w_d[1, :, 0 : 9 * C])
            nc.sync.dma_start(wt1[:, 9 * C : NCH * C], w_d[1, :, 9 * C : NCH * C])

            ta0 = img.tile([128, PP], bf16, name="Ta_0", tag="Ta_0")
            nc.scalar.dma_start(ta0[:], ta_d[0])
            Ta[0] = ta0

            t34_0 = img.tile([128, PP], bf16, name="T34_0", tag="T34_0")
            nc.scalar.dma_start(t34_0[:], tb34_d[0])
            T34[0] = t34_0

            t1_0 = img.tile([128, PP], bf16, name="T1_0", tag="T1_0")
            nc.scalar.dma_start(t1_0[:], tb1_d[0])
            T1[0] = t1_0

            # sample 1 inputs (arrive while sample 0 computes)
            ta1 = img.tile([128, PP], bf16, name="Ta_1", tag="Ta_1")
            nc.scalar.dma_start(ta1[:], ta_d[1])
            Ta[1] = ta1

            t34_1 = img.tile([128, PP], bf16, name="T34_1", tag="T34_1")
            nc.scalar.dma_start(t34_1[:], tb34_d[1])
            T34[1] = t34_1

            t1_1 = img.tile([128, PP], bf16, name="T1_1", tag="T1_1")
            nc.scalar.dma_start(t1_1[:], tb1_d[1])
            T1[1] = t1_1

            # --- PSUM tiles (8 banks, one per pixel block) ------------------
            psb = {}
            for s in range(S):
                for b in range(NB):
                    psb[(s, b)] = ps.tile([128, C], f32, name=f"ps_{s}_{b}",
                                          tag="ps")

            # --- PE warmup on zeros: bridge until first inputs arrive -------
            scr = cst.tile([128, C], bf16, name="scr", tag="scr")
            nc.vector.memset(scr[:], 0.0)
            for i, n in enumerate(WARMUP_NS):
                nc.tensor.matmul(psb[(0, 0)][:, 0:n], scr[:, 0:128], scr[:, 0:n],
                                 start=True, stop=True, skip_group_check=True)

            # --- matmul emission helpers ------------------------------------
            def lhsT(s, b, c):
                if c < 9:
                    dy, dx = TAPS[c]
                    v = Ta[s][:].rearrange("p (r q) -> p r q", q=PW)
                    return v[:, 4 * b + dy : 4 * b + dy + 4, dx : dx + 32]
                if c < 12:
                    j = c - 9
                    v = T34[s][:].rearrange("p (r q) -> p r q", q=PW)
                    return v[:, 4 * b : 4 * b + 4, j : j + 32]
                v = T1[s][:].rearrange("p (r q) -> p r q", q=PW)
                if c == 12:
                    return v[:, 4 * b + 2 : 4 * b + 6, 0:32]
                return v[0:64, 4 * b + 2 : 4 * b + 6, 2:34]

            def rhs(s, c):
                if c == 13:
                    return Wt[s][0:64, c * C : (c + 1) * C]
                return Wt[s][:, c * C : (c + 1) * C]

            def mm(s, b, c):
                nc.tensor.matmul(psb[(s, b)][:], lhsT(s, b, c), rhs(s, c),
                                 start=(c == 0), stop=(c == NCH - 1))

            # --- eviction + output DMA --------------------------------------
            stages = {}

            def evict(s, b):
                i = b // 2
                if b % 2 == 0:
                    stages[(s, i)] = oev.tile([128, 2 * C], f32,
                                              name=f"st_{s}_{i}", tag="st")
                dst = stages[(s, i)][:, (b % 2) * C : (b % 2 + 1) * C]
                if b % 2 == 0:
                    nc.vector.tensor_scalar_add(dst, psb[(s, b)][:], 0.0)
                else:
                    nc.scalar.copy(dst, psb[(s, b)][:])
                if b % 2 == 1:
                    st = stages[(s, i)][:].rearrange("p (b m) -> p b m", m=C)
                    if s == S - 1 and b == NB - 1:
                        # tail: last transfer on SP's HWDGE (shorter chain
                        # than SWDGE, and both are idle by now)
                        nc.sync.dma_start(out_d[s, :, 2 * i : 2 * i + 2, :], st)
                    else:
                        nc.gpsimd.dma_start(out_d[s, :, 2 * i : 2 * i + 2, :], st)

            # --- sample 0: chunk-sweep order (matches DMA arrival) ----------
            for c in range(4):            # needs W c0-c3 + Ta
                for b in range(NB):
                    mm(0, b, c)
            for c in range(4, 9):         # needs W c4-c8
                for b in range(NB):
                    mm(0, b, c)
            for b in range(NB):           # needs T34, T1, W c9-c13
                for c in range(9, NCH):
                    mm(0, b, c)
                evict(0, b)

            # --- sample 1: block-major ------------------------------------
            for b in range(NB):
                for c in range(NCH):
                    mm(1, b, c)
                evict(1, b)

    nc.compile()
    return nc


def get_module():
    if "nc" not in _cache:
        _cache["nc"] = _build_module()
    return _cache["nc"]


def _route(x, gate_w, gate_b):
    """Replicates the reference router in numpy fp32. Returns combine [B,E]."""
    pooled = x.mean(axis=(2, 3), dtype=np.float32)
    logits = pooled @ gate_w + gate_b
    z = logits - logits.max(axis=-1, keepdims=True)
    ez = np.exp(z)
    w = ez / ez.sum(axis=-1, keepdims=True)
    topi = np.argsort(-w, axis=-1, kind="stable")[:, :TOPK]
    topw = np.take_along_axis(w, topi, axis=-1)
    topw = topw / (topw.sum(-1, keepdims=True) + 1e-10)
    combine = np.zeros((B, E), np.float32)
    np.put_along_axis(combine, topi, topw, axis=-1)
    return combine


def make_in_maps(x, gate_w, gate_b, expert_w, expert_b):
    import ml_dtypes

    bf16 = ml_dtypes.bfloat16
    x = np.ascontiguousarray(np.asarray(x, np.float32))
    gate_w = np.asarray(gate_w, np.float32)
    gate_b = np.asarray(gate_b, np.float32)
    expert_w = np.asarray(expert_w, np.float32)
    expert_b = np.asarray(expert_b, np.float32)

    combine = _route(x, gate_w, gate_b)                       # [B,E]
    Wc = np.einsum("be,eoikl->boikl", combine, expert_w)      # [B,C,C,3,3]
    bc = combine @ expert_b                                   # [B,C]

    # Padded input images: [B, C, 34*34]
    xp = np.zeros((B, C, PW, PW), np.float32)
    xp[:, :, 1 : H + 1, 1 : W + 1] = x
    xp = xp.reshape(B, C, PP).astype(bf16)

    ta = xp[:, 0:128]                                         # [B,128,1156]
    img64 = xp[:, 128:192]                                    # [B,64,1156]

    # ch128-191 duplicated with the upper half pre-shifted (d=34 / d=1)
    tb34 = np.zeros((B, 128, PP), bf16)
    tb34[:, 0:64] = img64
    tb34[:, 64:128, 0 : PP - PW] = img64[:, :, PW:]
    tb1 = np.zeros((B, 128, PP), bf16)
    tb1[:, 0:64] = img64
    tb1[:, 64:128, 0 : PP - 1] = img64[:, :, 1:]

    # Moving weights, one [128, 192] slab per K-chunk:
    #   WT[b, t, i, o] = Wc[b, o, i, dy, dx]
    WT = Wc.transpose(0, 3, 4, 2, 1).reshape(B, 9, C, C)
    w = np.zeros((B, 128, NCH * C), np.float32)
    for c in range(9):                        # ch 0-127, tap c
        w[:, :, c * C : (c + 1) * C] = WT[:, c, 0:128, :]
    for j in range(3):                        # pairs (0,j)+(1,j), d=34
        c = 9 + j
        w[:, 0:64, c * C : (c + 1) * C] = WT[:, j, 128:192, :]
        w[:, 64:128, c * C : (c + 1) * C] = WT[:, 3 + j, 128:192, :]
    w[:, 0:64, 12 * C : 13 * C] = WT[:, 6, 128:192, :]        # pair (2,0)+(2,1), d=1
    w[:, 64:128, 12 * C : 13 * C] = WT[:, 7, 128:192, :]
    w[:, 0:64, 13 * C : 14 * C] = WT[:, 8, 128:192, :]        # single (2,2), K=64
    w = w.astype(bf16)

    in_maps = []
    for cidx in range(NCORES):
        b0 = S * cidx
        in_maps.append(
            {
                "ta": np.ascontiguousarray(ta[b0 : b0 + S]),
                "tb34": np.ascontiguousarray(tb34[b0 : b0 + S]),
                "tb1": np.ascontiguousarray(tb1[b0 : b0 + S]),
                "w": np.ascontiguousarray(w[b0 : b0 + S]),
            }
        )
    return in_maps, bc


def postprocess(dev_out, bc_rows):
    """[S, 128, NB, 192] device tensor + per-sample bias rows -> [S,C,H,W]."""
    dev = np.asarray(dev_out, np.float32)
    out = dev.transpose(0, 3, 2, 1).reshape(S, C, NB, 4, 32).reshape(S, C, H, W)
    return out + bc_rows[:, :, None, None]


def kernel(x, gate_w, gate_b, expert_w, expert_b):
    from concourse.bass_utils import run_bass_kernel_spmd

    nc = get_module()
    in_maps, bc = make_in_maps(x, gate_w, gate_b, expert_w, expert_b)
    res = run_bass_kernel_spmd(nc, in_maps, core_ids=list(range(NCORES)))
    out = np.empty((B, C, H, W), np.float32)
    for c in range(NCORES):
        b0 = S * c
        out[b0 : b0 + S] = postprocess(res.results[c]["out"], bc[b0 : b0 + S])
    return out
